# revision 11
# baseline (speedup 1.0000x reference)
"""Trainium2 Bass kernel for a 2-layer GCN (GCNConv -> ReLU -> GCNConv).

Strategy: partition nodes (dst) across 8 cores; replicate the small weights.
Aggregation = dma_gather of h'[src] rows + one-hot matmul scatter-add into
PSUM per 128-node dst block.  Exact fp32 math.

dma_gather uses int16 row indices, so the gather tables (h' and out1') are
addressed in 4 quarters of n_star/4 rows each; edges are binned by
(dst-chunk, src-quarter, dst-block) on the host so each (chunk, quarter)
range is one big gather.

Self-contained: hardcodes the full-problem shapes; host-side prep only
bins/pads edge lists (sharding) and transposes z.
"""

import os
import sys
import types

import numpy as np

# The trimmed container lacks antenv.axon_hooks; stub it so
# run_bass_kernel_spmd's trace path degrades gracefully instead of raising.
try:
    import antenv.axon_hooks  # noqa: F401
except (ImportError, ModuleNotFoundError):
    try:
        import antenv
        _stub = types.ModuleType("antenv.axon_hooks")
        _stub.get_axon_ntff_profile_hook = lambda: None
        sys.modules["antenv.axon_hooks"] = _stub
        antenv.axon_hooks = _stub
    except ImportError:
        pass

import concourse.bass as bass
import concourse.mybir as mybir
import concourse.tile as tile
from concourse import bacc
from concourse import bass_utils

P = 128
NQ = 4  # src-quarter count (int16 gather index limit: n_star/NQ < 32768)


# ----------------------------------------------------------------------------
# Host-side sharding prep
# ----------------------------------------------------------------------------

class EdgePlan:
    """Static structure of the binned edge lists (identical for all cores)."""

    def __init__(self, ncores, slice_sz, chunk_blocks):
        self.ncores = ncores
        self.slice_sz = slice_sz
        self.nblk = slice_sz // P
        self.n_star = ncores * slice_sz
        self.qsize = self.n_star // NQ
        self.chunk_blocks = chunk_blocks
        nch = -(-self.nblk // chunk_blocks)
        self.chunks = [list(range(c * chunk_blocks,
                                  min((c + 1) * chunk_blocks, self.nblk)))
                       for c in range(nch)]
        # filled by plan():
        self.tcnt = None        # [nch, NQ, nblk] tiles per group
        self.t0 = None          # [nch, NQ, nblk] absolute first tile of group
        self.TT = 0


def prep_edges(src, dst, ncores, slice_sz, chunk_blocks):
    """Bin edges (incl. self-loops, appended by caller) by
    (dst-chunk, src-quarter, dst-block).  Pads each group to a multiple of
    128 edges, uniform across cores.

    Returns (plan, gidx, dstl):
      gidx: [ncores, P, TT*8] int16  src index local to its quarter, in the
            dma_gather wrapped-16 layout (replicated across 8 groups of 16)
      dstl: [ncores, P, TT] f32      dst offset within block, -1 = pad
    """
    pl = EdgePlan(ncores, slice_sz, chunk_blocks)
    nblk, nch, qsize = pl.nblk, len(pl.chunks), pl.qsize

    blk = dst // P
    core = blk // nblk
    b = blk % nblk
    ch = b // chunk_blocks
    q = src // qsize
    key = ((core * nch + ch) * NQ + q) * nblk + b
    order = np.argsort(key, kind="stable")
    src_s = src[order]
    dst_s = dst[order]
    counts = np.bincount(key, minlength=ncores * nch * NQ * nblk)
    counts4 = counts.reshape(ncores, nch, NQ, nblk)
    tcnt = (-(-counts4 // P)).max(axis=0)  # [nch, NQ, nblk]

    # absolute tile offsets, in (ch, q, b-within-chunk) emission order
    t0 = np.zeros_like(tcnt)
    t = 0
    for c in range(nch):
        for qq in range(NQ):
            for bb in pl.chunks[c]:
                t0[c, qq, bb] = t
                t += int(tcnt[c, qq, bb])
    pl.tcnt, pl.t0, pl.TT = tcnt, t0, t
    TT = pl.TT

    bstart = np.zeros(ncores * nch * NQ * nblk + 1, dtype=np.int64)
    bstart[1:] = np.cumsum(counts)

    gsrc = np.zeros((ncores, TT * P), dtype=np.int16)
    dstl = np.full((ncores, TT * P), -1.0, dtype=np.float32)
    for cr in range(ncores):
        for c in range(nch):
            for qq in range(NQ):
                for bb in pl.chunks[c]:
                    k = ((cr * nch + c) * NQ + qq) * nblk + bb
                    s0, s1 = bstart[k], bstart[k + 1]
                    cnt = s1 - s0
                    if cnt == 0:
                        continue
                    off = int(t0[c, qq, bb]) * P
                    gsrc[cr, off:off + cnt] = (src_s[s0:s1] - qq * qsize
                                               ).astype(np.int16)
                    dstl[cr, off:off + cnt] = (dst_s[s0:s1] % P
                                               ).astype(np.float32)
    # dstl: edge f = t*128 + p  ->  [P, TT] (p, t)
    dstl = np.ascontiguousarray(dstl.reshape(ncores, TT, P).transpose(0, 2, 1))
    # gidx: edge f -> [f % 16, f // 16], replicated on 8 groups of 16 parts
    g16 = np.ascontiguousarray(
        gsrc.reshape(ncores, TT * 8, 16).transpose(0, 2, 1))  # [nc, 16, TT*8]
    gidx = np.tile(g16, (1, 8, 1))  # [nc, 128, TT*8]
    return pl, np.ascontiguousarray(gidx), dstl


# ----------------------------------------------------------------------------
# Device kernel builder
# ----------------------------------------------------------------------------

def build_kernel(pl, f_in, f_h, f_out, zchunk=16, debug=False):
    ncores, slice_sz, nblk = pl.ncores, pl.slice_sz, pl.nblk
    n_star, qsize, TT = pl.n_star, pl.qsize, pl.TT
    nt_h = n_star // P
    dt = mybir.dt

    # per-block group list: (abs t0, tiles), in emission order
    def block_groups(c, bb):
        out = []
        for qq in range(NQ):
            n = int(pl.tcnt[c, qq, bb])
            if n:
                out.append((int(pl.t0[c, qq, bb]), n))
        return out

    max_grp = max(max((n for _, n in block_groups(c, bb)), default=1)
                  for c in range(len(pl.chunks)) for bb in pl.chunks[c])
    chunk_tiles = [sum(int(pl.tcnt[c, qq, bb]) for qq in range(NQ)
                       for bb in pl.chunks[c]) for c in range(len(pl.chunks))]
    max_ct = max(chunk_tiles)

    nc = bacc.Bacc("TRN2", target_bir_lowering=False, debug=False,
                   num_devices=ncores)

    # --- I/O ---
    zT_d = nc.dram_tensor("zT", [f_in, n_star], dt.float32, kind="ExternalInput")
    W1_d = nc.dram_tensor("W1", [f_in, f_h], dt.float32, kind="ExternalInput")
    W2_d = nc.dram_tensor("W2", [f_h, f_out], dt.float32, kind="ExternalInput")
    b1b_d = nc.dram_tensor("b1b", [P, f_h], dt.float32, kind="ExternalInput")
    b2b_d = nc.dram_tensor("b2b", [P, f_out], dt.float32, kind="ExternalInput")
    iota_d = nc.dram_tensor("iota", [P, P], dt.float32, kind="ExternalInput")
    gidx_d = nc.dram_tensor("gidx", [P, TT * 8], dt.int16, kind="ExternalInput")
    dstl_d = nc.dram_tensor("dstl", [P, TT], dt.float32, kind="ExternalInput")
    y_d = nc.dram_tensor("y", [slice_sz, f_out], dt.float32, kind="ExternalOutput")

    # --- internal DRAM ---
    ag_space = "Shared" if ncores > 4 else "Local"
    hp_d = nc.dram_tensor("hp", [n_star, f_h], dt.float32)
    dga_in = nc.dram_tensor("dga_in", [P * nblk], dt.float32)
    dga_out = nc.dram_tensor("dga_out", [ncores * P * nblk], dt.float32,
                             addr_space=ag_space)
    o1p_d = nc.dram_tensor("o1p", [slice_sz, f_h], dt.float32)
    o1f_d = nc.dram_tensor("o1f", [n_star, f_h], dt.float32, addr_space=ag_space)

    if debug:
        dbg_deg = nc.dram_tensor("dbg_deg", [P, nblk], dt.float32,
                                 kind="ExternalOutput")
        dbg_dinvf = nc.dram_tensor("dbg_dinvf", [P, nt_h], dt.float32,
                                   kind="ExternalOutput")
        dbg_hp = nc.dram_tensor("dbg_hp", [n_star, f_h], dt.float32,
                                kind="ExternalOutput")
        dbg_o1p = nc.dram_tensor("dbg_o1p", [slice_sz, f_h], dt.float32,
                                 kind="ExternalOutput")
        dbg_g0 = nc.dram_tensor("dbg_g0", [P, chunk_tiles[0] * f_h], dt.float32,
                                kind="ExternalOutput")

    groups = [list(range(ncores))]

    def build_S(sp, gt0, gn):
        """One-hot matrix S [P(edges), gn*P] for group tiles [gt0, gt0+gn)."""
        s_t = sp.tile([P, max_grp * P], dt.float32, tag="sblk")
        out = s_t[:, :gn * P].rearrange("p (t j) -> p t j", t=gn)
        in0 = iota_t[:].unsqueeze(1).to_broadcast([P, gn, P])
        in1 = dstl_t[:, gt0:gt0 + gn].unsqueeze(2).to_broadcast([P, gn, P])
        nc.vector.tensor_tensor(out=out, in0=in0, in1=in1,
                                op=mybir.AluOpType.is_equal)
        return s_t

    def gather_chunk(gp, ip, c, table_d):
        """dma_gather the whole chunk c from table_d (one call per quarter)."""
        ct = chunk_tiles[c]
        ct0 = min(int(pl.t0[c, qq, bb]) for qq in range(NQ)
                  for bb in pl.chunks[c])
        gbuf = gp.tile([P, max_ct * f_h], dt.float32, tag="gbuf")
        gix = ip.tile([P, max_ct * 8], dt.int16, tag="gix")
        nc.sync.dma_start(gix[:, :ct * 8], gidx_d[:, ct0 * 8:(ct0 + ct) * 8])
        for qq in range(NQ):
            qt = sum(int(pl.tcnt[c, qq, bb]) for bb in pl.chunks[c])
            if qt == 0:
                continue
            qt0 = min(int(pl.t0[c, qq, bb]) for bb in pl.chunks[c]
                      if pl.tcnt[c, qq, bb]) - ct0  # chunk-relative
            n = qt * P
            nc.gpsimd.dma_gather(
                out_ap=gbuf[:, qt0 * f_h:(qt0 + qt) * f_h].rearrange(
                    "p (t f) -> p t f", t=qt),
                in_ap=table_d[qq * qsize:(qq + 1) * qsize, :],
                idxs_ap=gix[:, qt0 * 8:(qt0 + qt) * 8],
                num_idxs=n,
                num_idxs_reg=n,
                elem_size=f_h,
                single_packet=False,
            )
        return gbuf, ct0

    with tile.TileContext(nc) as tc:
        with tc.tile_pool(name="persist", bufs=1) as pp:
            iota_t = pp.tile([P, P], dt.float32)
            W1_t = pp.tile([f_in, f_h], dt.float32)
            W2_t = pp.tile([f_h, f_out], dt.float32)
            b1b_t = pp.tile([P, f_h], dt.float32)
            b2b_t = pp.tile([P, f_out], dt.float32)
            dstl_t = pp.tile([P, TT], dt.float32)
            ones_t = pp.tile([P, 1], dt.float32)
            deg_t = pp.tile([P, nblk], dt.float32)
            dinvl_t = pp.tile([P, nblk], dt.float32)
            dinvf_t = pp.tile([P, nt_h], dt.float32)

            nc.sync.dma_start(iota_t[:], iota_d[:])
            nc.sync.dma_start(W1_t[:], W1_d[:])
            nc.sync.dma_start(W2_t[:], W2_d[:])
            nc.sync.dma_start(b1b_t[:], b1b_d[:])
            nc.sync.dma_start(b2b_t[:], b2b_d[:])
            nc.sync.dma_start(dstl_t[:], dstl_d[:])
            nc.gpsimd.memset(ones_t[:], 1.0)

            # ---------------- Phase A: degree ----------------
            with tc.tile_pool(name="pa_s", bufs=3) as sp, \
                 tc.tile_pool(name="pa_ps", bufs=4, space="PSUM") as psp:
                for c in range(len(pl.chunks)):
                    for bb in pl.chunks[c]:
                        grps = block_groups(c, bb)
                        ntile = sum(n for _, n in grps)
                        dps = psp.tile([P, 1], dt.float32, tag="degps")
                        k = 0
                        for gt0, gn in grps:
                            s_t = build_S(sp, gt0, gn)
                            for t in range(gn):
                                nc.tensor.matmul(
                                    dps[:], lhsT=s_t[:, t * P:(t + 1) * P],
                                    rhs=ones_t[:],
                                    start=(k == 0), stop=(k == ntile - 1))
                                k += 1
                        nc.vector.tensor_copy(deg_t[:, bb:bb + 1], dps[:])

            nc.sync.dma_start(
                dga_in[:].rearrange("(p b) -> p b", p=P), deg_t[:])
            nc.gpsimd.collective_compute(
                "AllGather", mybir.AluOpType.bypass, replica_groups=groups,
                ins=[dga_in[:].opt()], outs=[dga_out[:].opt()])
            dga_out_v = dga_out[:].rearrange("(c p b) -> c p b", c=ncores, p=P)
            for cr in range(ncores):
                nc.sync.dma_start(dinvf_t[:, cr * nblk:(cr + 1) * nblk],
                                  dga_out_v[cr])
            # dinv = sqrt(1/deg)  (deg >= 1 always: self loops)
            nc.vector.reciprocal(dinvf_t[:], dinvf_t[:])
            nc.scalar.sqrt(dinvf_t[:], dinvf_t[:])
            nc.vector.reciprocal(dinvl_t[:], deg_t[:])
            nc.scalar.sqrt(dinvl_t[:], dinvl_t[:])
            if debug:
                nc.sync.dma_start(dbg_deg[:], deg_t[:])
                nc.sync.dma_start(dbg_dinvf[:], dinvf_t[:])

            # ---------------- Phase B: h' = dinv * (z @ W1) ----------------
            with tc.tile_pool(name="pb_z", bufs=2) as zp, \
                 tc.tile_pool(name="pb_h", bufs=4) as hsp, \
                 tc.tile_pool(name="pb_ps", bufs=4, space="PSUM") as psp:
                n0 = 0
                while n0 < nt_h:
                    zc = min(zchunk, nt_h - n0)
                    zbuf = zp.tile([f_in, zchunk * P], dt.float32, tag="zbuf")
                    nc.sync.dma_start(zbuf[:, :zc * P],
                                      zT_d[:, n0 * P:(n0 + zc) * P])
                    for t in range(zc):
                        g = n0 + t
                        hps = psp.tile([P, f_h], dt.float32, tag="hps")
                        nc.tensor.matmul(hps[:], lhsT=zbuf[:, t * P:(t + 1) * P],
                                         rhs=W1_t[:], start=True, stop=True)
                        hsb = hsp.tile([P, f_h], dt.float32, tag="hsb")
                        nc.scalar.activation(hsb[:], hps[:],
                                             mybir.ActivationFunctionType.Copy,
                                             scale=dinvf_t[:, g:g + 1])
                        nc.sync.dma_start(hp_d[g * P:(g + 1) * P, :], hsb[:])
                        if debug:
                            nc.sync.dma_start(
                                dbg_hp[g * P:(g + 1) * P, :], hsb[:])
                    n0 += zc

            # ---------------- Phase C: layer-1 aggregation ----------------
            with tc.tile_pool(name="pc_g", bufs=2) as gp, \
                 tc.tile_pool(name="pc_i", bufs=2) as ip, \
                 tc.tile_pool(name="pc_s", bufs=3) as sp, \
                 tc.tile_pool(name="pc_e", bufs=3) as ep, \
                 tc.tile_pool(name="pc_ps", bufs=4, space="PSUM") as psp:
                for c in range(len(pl.chunks)):
                    gbuf, ct0 = gather_chunk(gp, ip, c, hp_d)
                    if debug and c == 0:
                        nc.sync.dma_start(
                            dbg_g0[:], gbuf[:, :chunk_tiles[0] * f_h])
                    for bb in pl.chunks[c]:
                        grps = block_groups(c, bb)
                        ntile = sum(n for _, n in grps)
                        aps = psp.tile([P, f_h], dt.float32, tag="aggps")
                        k = 0
                        for gt0, gn in grps:
                            s_t = build_S(sp, gt0, gn)
                            for t in range(gn):
                                gcol = (gt0 - ct0 + t) * f_h
                                nc.tensor.matmul(
                                    aps[:], lhsT=s_t[:, t * P:(t + 1) * P],
                                    rhs=gbuf[:, gcol:gcol + f_h],
                                    start=(k == 0), stop=(k == ntile - 1))
                                k += 1
                        # out1' = dinv * relu(dinv * agg + b1)
                        t1 = ep.tile([P, f_h], dt.float32, tag="e1")
                        nc.scalar.activation(t1[:], aps[:],
                                             mybir.ActivationFunctionType.Copy,
                                             scale=dinvl_t[:, bb:bb + 1])
                        t2 = ep.tile([P, f_h], dt.float32, tag="e2")
                        nc.vector.tensor_add(t2[:], t1[:], b1b_t[:])
                        t3 = ep.tile([P, f_h], dt.float32, tag="e3")
                        nc.scalar.activation(t3[:], t2[:],
                                             mybir.ActivationFunctionType.Relu,
                                             scale=dinvl_t[:, bb:bb + 1])
                        nc.sync.dma_start(o1p_d[bb * P:(bb + 1) * P, :], t3[:])
                        if debug:
                            nc.sync.dma_start(
                                dbg_o1p[bb * P:(bb + 1) * P, :], t3[:])

            nc.gpsimd.collective_compute(
                "AllGather", mybir.AluOpType.bypass, replica_groups=groups,
                ins=[o1p_d[:].opt()], outs=[o1f_d[:].opt()])

            # ---------------- Phase D: layer-2 aggregation + W2 ----------------
            with tc.tile_pool(name="pd_g", bufs=2) as gp, \
                 tc.tile_pool(name="pd_i", bufs=2) as ip, \
                 tc.tile_pool(name="pd_s", bufs=3) as sp, \
                 tc.tile_pool(name="pd_e", bufs=3) as ep, \
                 tc.tile_pool(name="pd_u", bufs=3) as up, \
                 tc.tile_pool(name="pd_ps", bufs=2, space="PSUM") as psp, \
                 tc.tile_pool(name="pd_ps2", bufs=2, space="PSUM") as psp2:
                for c in range(len(pl.chunks)):
                    gbuf, ct0 = gather_chunk(gp, ip, c, o1f_d)
                    for bb in pl.chunks[c]:
                        grps = block_groups(c, bb)
                        ntile = sum(n for _, n in grps)
                        # psum2T [f_h, P(dst)] = sum_t Hg_t^T @ S_t
                        aps = psp.tile([f_h, P], dt.float32, tag="agg2ps")
                        k = 0
                        for gt0, gn in grps:
                            s_t = build_S(sp, gt0, gn)
                            for t in range(gn):
                                gcol = (gt0 - ct0 + t) * f_h
                                nc.tensor.matmul(
                                    aps[:], lhsT=gbuf[:, gcol:gcol + f_h],
                                    rhs=s_t[:, t * P:(t + 1) * P],
                                    start=(k == 0), stop=(k == ntile - 1))
                                k += 1
                        u = up.tile([f_h, P], dt.float32, tag="u2")
                        nc.vector.tensor_copy(u[:], aps[:])
                        xps = psp2.tile([P, f_out], dt.float32, tag="xps")
                        nc.tensor.matmul(xps[:], lhsT=u[:], rhs=W2_t[:],
                                         start=True, stop=True)
                        # y = dinv * (agg2 @ W2) + b2
                        x1 = ep.tile([P, f_out], dt.float32, tag="x1")
                        nc.scalar.activation(x1[:], xps[:],
                                             mybir.ActivationFunctionType.Copy,
                                             scale=dinvl_t[:, bb:bb + 1])
                        x2 = ep.tile([P, f_out], dt.float32, tag="x2")
                        nc.vector.tensor_add(x2[:], x1[:], b2b_t[:])
                        nc.sync.dma_start(y_d[bb * P:(bb + 1) * P, :], x2[:])

    nc.compile()
    return nc


# ----------------------------------------------------------------------------
# Host wrapper
# ----------------------------------------------------------------------------

def make_inputs(z, edge_index, W1, b1, W2, b2, ncores, slice_sz,
                chunk_blocks=4):
    n = z.shape[0]
    n_star = slice_sz * ncores
    f_in = z.shape[1]

    src = np.asarray(edge_index[0], dtype=np.int64)
    dst = np.asarray(edge_index[1], dtype=np.int64)
    loops = np.arange(n_star, dtype=np.int64)
    src = np.concatenate([src, loops])
    dst = np.concatenate([dst, loops])

    pl, gidx, dstl = prep_edges(src, dst, ncores, slice_sz, chunk_blocks)

    zT = np.zeros((f_in, n_star), dtype=np.float32)
    zT[:, :n] = np.asarray(z, dtype=np.float32).T
    b1b = np.tile(np.asarray(b1, dtype=np.float32)[None, :], (P, 1))
    b2b = np.tile(np.asarray(b2, dtype=np.float32)[None, :], (P, 1))
    iota = np.tile(np.arange(P, dtype=np.float32)[None, :], (P, 1))

    common = {
        "zT": np.ascontiguousarray(zT),
        "W1": np.ascontiguousarray(np.asarray(W1, dtype=np.float32)),
        "W2": np.ascontiguousarray(np.asarray(W2, dtype=np.float32)),
        "b1b": np.ascontiguousarray(b1b),
        "b2b": np.ascontiguousarray(b2b),
        "iota": np.ascontiguousarray(iota),
    }
    in_maps = []
    for c in range(ncores):
        m = dict(common)
        m["gidx"] = gidx[c]
        m["dstl"] = dstl[c]
        in_maps.append(m)
    return pl, in_maps


_CACHE = {}


def kernel(z, edge_index, W1, b1, W2, b2):
    NCORES = 8
    N = 100000
    SLICE = 12544  # 98 blocks of 128; 8*12544 = 100352 >= N

    pl, in_maps = make_inputs(z, edge_index, W1, b1, W2, b2, NCORES, SLICE)

    ck = (tuple(pl.tcnt.ravel().tolist()), z.shape, edge_index.shape)
    if ck not in _CACHE:
        _CACHE[ck] = build_kernel(pl, f_in=z.shape[1], f_h=W1.shape[1],
                                  f_out=W2.shape[1])
    nc = _CACHE[ck]

    trace = bool(int(os.environ.get("KERNEL_TRACE", "0")))
    res = bass_utils.run_bass_kernel_spmd(
        nc, in_maps, core_ids=list(range(NCORES)), trace=trace)
    if trace and res.exec_time_ns is not None:
        print(f"HW exec time: {res.exec_time_ns} ns")
        kernel.last_exec_time_ns = res.exec_time_ns
        kernel.last_trace = res.instructions_and_trace
    y = np.concatenate([r["y"] for r in res.results], axis=0)[:N]
    return np.ascontiguousarray(y, dtype=np.float32)
